# revision 15
# baseline (speedup 1.0000x reference)
"""Binary-tree gated-expert MoE (root -> 2 mid -> 4 leaf experts) on 8 trn2 cores.

Strategy: expert-parallel dispatch by leaf index. Tokens are grouped on the
host by their 2-bit routing path (leaf = 2*bit0 + bit1); each of the 8
NeuronCores processes one contiguous chunk of one leaf's tokens (cores are
apportioned to leaves proportionally to token counts, 2 cores/leaf in the
balanced case). A core then runs 3 chained dense [C,2048]x[2048,2048] layers
(root W0, mid W1[bit0], leaf W2[leaf]) with relu+bias, entirely on-chip.

Device kernel keeps activations transposed ([D, tokens] feature-major) so each
layer's matmul output (PSUM [fout, tok]) is directly the next layer's rhs.
Matmuls run in fp16 (same TensorE rate as bf16, 8x finer mantissa) with fp32
PSUM accumulation; weights are streamed from HBM as pre-tiled [16, 128, 2048]
stripes and used as the stationary operand.

Schedule notes (from NTFF trace analysis): the kernel is TensorE-bound at
~1.02 cycles/column, so the only wins are at the edges. The first m-pair's
weight stripes stream as interleaved k-range slices so the pair-0 k-loop
(which trickles behind the x input DMA) never waits on stripe m=1; the output
is fp16 (harness tolerance 2e-2, fp16 adds ~2e-4) to halve output DMA; and
the last m-pair's epilogues are chunked column-wise across ACT/DVE and both
DMA rings so the post-matmul tail is short.
"""

import numpy as np
from contextlib import ExitStack

import concourse.bass as bass
from concourse import bacc, mybir, tile
from concourse.bass_utils import run_bass_kernel_spmd

D = 2048
PT = 128           # partition tile
KT = D // PT       # 16 contraction tiles per layer
MT = D // PT       # 16 output-feature tiles per layer
N_CORES = 8

F32 = mybir.dt.float32
F16 = mybir.dt.float16
NP_F16 = np.float16

# cache of compiled bass programs keyed by padded capacity C
_compiled = {}
# stash of the last run's results so a harness can inspect exec_time_ns
last_results = None


def _prep_weight(W):
    """[D, D] -> [MT, 128, D] fp16: stripe m holds W[:, m*128:(m+1)*128]
    rearranged so partition p = contraction row within k-chunk, and the free
    dim is (k, fout-col) — i.e. out[m, p, k*128 + c] = W[k*128 + p, m*128 + c].
    Each [128, 2048] stripe then DMAs contiguously into SBUF and its k-th
    [128, 128] column block is exactly the lhsT (stationary) matmul operand."""
    W4 = W.reshape(KT, PT, MT, PT)
    return np.ascontiguousarray(
        W4.transpose(2, 1, 0, 3).reshape(MT, PT, D).astype(NP_F16)
    )


def _prep_bias(b0, b1e, b2l):
    """three [D] biases -> [128, 3*MT] f32 where column li*MT + m holds
    bias[li][m*128 : (m+1)*128] along partitions."""
    cols = []
    for b in (b0, b1e, b2l):
        cols.append(b.reshape(MT, PT).T)  # [128, MT]
    return np.ascontiguousarray(np.concatenate(cols, axis=1).astype(np.float32))


def _tiling(maxg):
    """Pick (TN, NT, C): NT token tiles, first NT-1 of width TN (<= 512, one
    PSUM bank of fp32) and a final tile of C - (NT-1)*TN, with C = maxg
    exactly (no padding beyond the max per-core group size)."""
    maxg = max(maxg, 256)
    NT = -(-maxg // 512)
    TN = -(-maxg // NT)
    return TN, NT, maxg


def _build(C, TN, NT):
    """Build + compile the 3-layer SPMD program for per-core capacity C.

    Layer-1 matmuls must consume the 16 k-chunks of the input as they stream
    in, so the m loop runs in pairs (6 PSUM tiles live per pair, 8 banks
    total): each pair's k-loop trickles behind the input DMA instead of one
    m-tile waiting for the entire input. Weight stripes ride the scalar
    (qActDynamicHW) DMA ring so they never queue behind the big input
    transfers on the sync (qSPDynamicHW) ring."""
    widths = [TN] * (NT - 1) + [C - (NT - 1) * TN]
    starts = [i * TN for i in range(NT)]

    nc = bacc.Bacc(
        "TRN2",
        target_bir_lowering=False,
        debug=False,
        enable_asserts=False,
        num_devices=N_CORES,
    )
    xT = nc.dram_tensor("xT", [D, C], F16, kind="ExternalInput").ap()
    w0 = nc.dram_tensor("w0", [MT, PT, D], F16, kind="ExternalInput").ap()
    w1 = nc.dram_tensor("w1", [MT, PT, D], F16, kind="ExternalInput").ap()
    w2 = nc.dram_tensor("w2", [MT, PT, D], F16, kind="ExternalInput").ap()
    bias = nc.dram_tensor("bias", [PT, 3 * MT], F32, kind="ExternalInput").ap()
    yT = nc.dram_tensor("yT", [D, C], F16, kind="ExternalOutput").ap()

    with tile.TileContext(nc) as tc, ExitStack() as ctx:
        wpool = ctx.enter_context(tc.tile_pool(name="w", bufs=4))
        hpool = ctx.enter_context(tc.tile_pool(name="h", bufs=1))
        pspool = ctx.enter_context(tc.tile_pool(name="ps", bufs=8, space="PSUM"))
        opool = ctx.enter_context(tc.tile_pool(name="o", bufs=8))
        cpool = ctx.enter_context(tc.tile_pool(name="c", bufs=1))

        hA = hpool.tile([PT, KT, C], F16, tag="hA")
        hB = hpool.tile([PT, KT, C], F16, tag="hB")

        # All early DMAs round-robin across the shared SDMA engines at packet
        # granularity, so emission order ~= bandwidth share. The first matmul
        # needs stripe (w0, m=0) k=0 + x chunk 0; the pair-0 k-loop then
        # consumes (m0,k)+(m1,k) every ~0.9us, while x chunks land every
        # ~1.9us. Stream the two stripes as interleaved k-range slices so
        # stripe m=1's early k-slices don't queue behind all of stripe m=0
        # (a full stripe is ~3.8us of queue time; a late m1 k=1 slice stalls
        # the PE at the start of the k-loop).
        wts0 = []
        for m in (0, 1):
            wt = wpool.tile([PT, D], F16, tag="wt", name=f"wt0_{m}")
            nc.scalar.dma_start(wt[:, 0:PT], w0[m, :, 0:PT])
            wts0.append(wt)
        nc.sync.dma_start(hA[:, 0, 0:TN], xT[0:PT, 0:TN])
        for ks in range(1, KT, 4):  # k-groups 1-4, 5-8, 9-12, 13-15
            ke = min(ks + 4, KT)
            for m in (0, 1):
                nc.scalar.dma_start(
                    wts0[m][:, ks * PT : ke * PT], w0[m, :, ks * PT : ke * PT]
                )
        if TN < C:
            nc.gpsimd.dma_start(hA[:, 0, TN:C], xT[0:PT, TN:C])
        # The early phase is x-delivery-bound. Column-split every remaining
        # chunk across the sync and gpsimd rings (scalar carries the weight
        # stripes); the sync ring measures persistently ~30% slower than
        # gpsimd (~100 vs ~140 GB/s), so give it 42% of the columns. Pair-0's
        # end time tracks the arrival of its last chunk.
        CH = (42 * C // 100) & ~1
        for k in range(1, KT):
            nc.sync.dma_start(hA[:, k, 0:CH], xT[k * PT : (k + 1) * PT, 0:CH])
            nc.gpsimd.dma_start(hA[:, k, CH:C], xT[k * PT : (k + 1) * PT, CH:C])
        bias_sb = cpool.tile([PT, 3 * MT], F32)
        nc.scalar.dma_start(bias_sb[:], bias[:])

        def relu_bias(out_ap, ps_ap, b_ap, on_dve):
            if on_dve:
                nc.vector.tensor_scalar(
                    out_ap, ps_ap, b_ap, 0.0,
                    mybir.AluOpType.add, mybir.AluOpType.max,
                )
            else:
                nc.scalar.activation(
                    out_ap, ps_ap,
                    mybir.ActivationFunctionType.Relu, bias=b_ap,
                )

        layers = [(w0, 0, hA, hB), (w1, 1, hB, hA), (w2, 2, hA, None)]
        for w_dram, li, h_in, h_out in layers:
            for mp in range(MT // 2):
                ms = (2 * mp, 2 * mp + 1)
                if li == 0 and mp == 0:
                    wts = wts0
                else:
                    wts = []
                    halves = 2 if (li == 0 and mp == 1) else 1
                    for m in ms:
                        wt = wpool.tile([PT, D], F16, tag="wt", name=f"wt{li}_{m}")
                        wts.append(wt)
                    # pair 1 of layer 0 still races the pair-0 k-loop;
                    # interleave its two stripes as halves.
                    for hv in range(halves):
                        lo = hv * (D // halves)
                        hi = lo + D // halves
                        for mi, m in enumerate(ms):
                            nc.scalar.dma_start(
                                wts[mi][:, lo:hi], w_dram[m, :, lo:hi]
                            )
                pss = {
                    (m, n): pspool.tile([PT, TN], F32, tag="ps", name=f"ps{li}_{m}_{n}")
                    for m in ms
                    for n in range(NT)
                }

                def epilogue(mi, m, n):
                    n0, w = starts[n], widths[n]
                    b_ap = bias_sb[:, li * MT + m : li * MT + m + 1]
                    # alternate ACT/DVE so epilogues drain on two engines
                    on_dve = (n + mi) % 2 == 1
                    if h_out is not None:
                        relu_bias(
                            h_out[:, m, n0 : n0 + w], pss[(m, n)][:, :w],
                            b_ap, on_dve,
                        )
                    else:
                        # final layer: emit fp16 output tiles (harness
                        # tolerance is 2e-2; fp16 adds ~2e-4 and halves the
                        # output DMA, shortening the post-matmul tail)
                        ot = opool.tile([PT, w], F16, tag="ot", name=f"ot{m}_{n}")
                        relu_bias(ot[:], pss[(m, n)][:, :w], b_ap, on_dve)
                        if m == MT - 1 and n == NT - 1:
                            # very last tile: one epilogue op (splitting it
                            # false-serializes on the shared PSUM tile), then
                            # two half DMAs on separate rings so the final
                            # transfer is half-depth
                            h1 = w // 2
                            nc.scalar.dma_start(
                                yT[m * PT : (m + 1) * PT, n0 : n0 + h1],
                                ot[:, :h1],
                            )
                            nc.gpsimd.dma_start(
                                yT[m * PT : (m + 1) * PT, n0 + h1 : n0 + w],
                                ot[:, h1:w],
                            )
                        else:
                            # rotate output DMAs over all three rings so the
                            # final pairs' transfers drain before the kernel
                            # end instead of backing up on two queues
                            dma_eng = (nc.scalar, nc.sync, nc.gpsimd)[(m * NT + n) % 3]
                            dma_eng.dma_start(
                                yT[m * PT : (m + 1) * PT, n0 : n0 + w], ot[:]
                            )

                if li == 0:
                    # k-outer: consume the streaming input chunks as they land
                    for k in range(KT):
                        for mi, m in enumerate(ms):
                            for n in range(NT):
                                n0, w = starts[n], widths[n]
                                nc.tensor.matmul(
                                    pss[(m, n)][:, :w],
                                    wts[mi][:, k * PT : (k + 1) * PT],
                                    h_in[:, k, n0 : n0 + w],
                                    start=(k == 0),
                                    stop=(k == KT - 1),
                                    skip_group_check=True,
                                )
                    for mi, m in enumerate(ms):
                        for n in range(NT):
                            epilogue(mi, m, n)
                else:
                    # inputs resident: k-inner per tile, so each tile's
                    # epilogue (and final-layer out-DMA) fires as soon as its
                    # accumulation completes — the kernel tail drains one
                    # tile, not six
                    for mi, m in enumerate(ms):
                        for n in range(NT):
                            n0, w = starts[n], widths[n]
                            for k in range(KT):
                                nc.tensor.matmul(
                                    pss[(m, n)][:, :w],
                                    wts[mi][:, k * PT : (k + 1) * PT],
                                    h_in[:, k, n0 : n0 + w],
                                    start=(k == 0),
                                    stop=(k == KT - 1),
                                )
                            epilogue(mi, m, n)
    nc.compile()
    return nc


def _apportion_cores(counts):
    """Assign 8 cores to 4 leaves ~proportionally to token counts.
    Returns list of core counts per leaf (sums to N_CORES; 0 only for empty
    leaves). Greedy: repeatedly hand a core to the leaf with max load/core."""
    alive = [l for l in range(4) if counts[l] > 0]
    n = {l: 1 for l in alive}
    for _ in range(N_CORES - len(alive)):
        l = max(alive, key=lambda l: counts[l] / n[l])
        n[l] += 1
    return [n.get(l, 0) for l in range(4)]


def kernel(x, W0, b0, W1, b1, W2, b2, path_mask):
    global last_results
    x = np.asarray(x, dtype=np.float32)
    path_mask = np.asarray(path_mask)
    W0, b0, W1, b1, W2, b2 = (
        np.asarray(a, dtype=np.float32) for a in (W0, b0, W1, b1, W2, b2)
    )
    B = x.shape[0]

    bit0 = path_mask[:, 0].astype(np.int64)
    bit1 = path_mask[:, 1].astype(np.int64)
    leaf = 2 * bit0 + bit1
    order = np.argsort(leaf, kind="stable")
    counts = np.bincount(leaf, minlength=4)

    per_leaf = _apportion_cores(counts)
    # contiguous chunks of the leaf-sorted order per core
    groups = []      # list of (leaf, index-array) per core
    start = 0
    for l in range(4):
        cnt = int(counts[l])
        tok = order[start : start + cnt]
        start += cnt
        nl = per_leaf[l]
        if nl == 0:
            continue
        bounds = [round(i * cnt / nl) for i in range(nl + 1)]
        for i in range(nl):
            groups.append((l, tok[bounds[i] : bounds[i + 1]]))
    while len(groups) < N_CORES:  # only if some leaf was empty and slots remain
        groups.append((0, np.zeros(0, dtype=np.int64)))

    maxg = max(len(g[1]) for g in groups)
    TN, NT, C = _tiling(maxg)

    if C not in _compiled:
        _compiled[C] = _build(C, TN, NT)
    nc = _compiled[C]

    w_prepped = {}  # cache per (matrix id)
    def wp(tag, W):
        if tag not in w_prepped:
            w_prepped[tag] = _prep_weight(W)
        return w_prepped[tag]

    xb = x.astype(NP_F16)
    in_maps = []
    for l, tok in groups:
        xTg = np.zeros((D, C), dtype=NP_F16)
        if len(tok):
            xTg[:, : len(tok)] = xb[tok].T
        in_maps.append(
            {
                "xT": xTg,
                "w0": wp("w0", W0),
                "w1": wp(("w1", l // 2), W1[l // 2]),
                "w2": wp(("w2", l), W2[l]),
                "bias": _prep_bias(b0, b1[l // 2], b2[l]),
            }
        )

    last_results = run_bass_kernel_spmd(nc, in_maps, core_ids=list(range(N_CORES)))

    y = np.empty((B, D), dtype=np.float32)
    for (l, tok), res in zip(groups, last_results.results):
        if len(tok):
            y[tok] = res["yT"][:, : len(tok)].T.astype(np.float32)
    return y


# revision 20
# speedup vs baseline: 1.0187x; 1.0187x over previous
"""Binary-tree gated-expert MoE (root -> 2 mid -> 4 leaf experts) on 8 trn2 cores.

Strategy: expert-parallel dispatch by leaf index. Tokens are grouped on the
host by their 2-bit routing path (leaf = 2*bit0 + bit1); each of the 8
NeuronCores processes one contiguous chunk of one leaf's tokens (cores are
apportioned to leaves proportionally to token counts, 2 cores/leaf in the
balanced case). A core then runs 3 chained dense [C,2048]x[2048,2048] layers
(root W0, mid W1[bit0], leaf W2[leaf]) with relu+bias, entirely on-chip.

Device kernel keeps activations transposed ([D, tokens] feature-major) so each
layer's matmul output (PSUM [fout, tok]) is directly the next layer's rhs.
Matmuls run in fp16 (same TensorE rate as bf16, 8x finer mantissa) with fp32
PSUM accumulation; weights are streamed from HBM as pre-tiled [16, 128, 2048]
stripes and used as the stationary operand.

Schedule notes (from NTFF trace analysis): the kernel is TensorE-bound at
~1.02 cycles/column, so the only wins are at the edges. The first m-pair's
weight stripes stream as interleaved k-range slices so the pair-0 k-loop
(which trickles behind the x input DMA) never waits on stripe m=1; the output
is fp16 (harness tolerance 2e-2, fp16 adds ~2e-4) to halve output DMA; and
the last m-pair's epilogues are chunked column-wise across ACT/DVE and both
DMA rings so the post-matmul tail is short.
"""

import numpy as np
from contextlib import ExitStack

import concourse.bass as bass
from concourse import bacc, mybir, tile
from concourse.bass_utils import run_bass_kernel_spmd

D = 2048
PT = 128           # partition tile
KT = D // PT       # 16 contraction tiles per layer
MT = D // PT       # 16 output-feature tiles per layer
N_CORES = 8

F32 = mybir.dt.float32
F16 = mybir.dt.float16
NP_F16 = np.float16

# cache of compiled bass programs keyed by padded capacity C
_compiled = {}
# stash of the last run's results so a harness can inspect exec_time_ns
last_results = None


def _prep_weight(W):
    """[D, D] -> [MT, 128, D] fp16: stripe m holds W[:, m*128:(m+1)*128]
    rearranged so partition p = contraction row within k-chunk, and the free
    dim is (k, fout-col) — i.e. out[m, p, k*128 + c] = W[k*128 + p, m*128 + c].
    Each [128, 2048] stripe then DMAs contiguously into SBUF and its k-th
    [128, 128] column block is exactly the lhsT (stationary) matmul operand."""
    W4 = W.reshape(KT, PT, MT, PT)
    return np.ascontiguousarray(
        W4.transpose(2, 1, 0, 3).reshape(MT, PT, D).astype(NP_F16)
    )


def _prep_bias(b0, b1e, b2l):
    """three [D] biases -> [128, 3*MT] f32 where column li*MT + m holds
    bias[li][m*128 : (m+1)*128] along partitions."""
    cols = []
    for b in (b0, b1e, b2l):
        cols.append(b.reshape(MT, PT).T)  # [128, MT]
    return np.ascontiguousarray(np.concatenate(cols, axis=1).astype(np.float32))


def _tiling(maxg):
    """Pick (TN, NT, C): NT token tiles, first NT-1 of width TN (<= 512, one
    PSUM bank of fp32) and a final tile of C - (NT-1)*TN, with C = maxg
    exactly (no padding beyond the max per-core group size)."""
    maxg = max(maxg, 256)
    NT = -(-maxg // 512)
    TN = -(-maxg // NT)
    return TN, NT, maxg


def _build(C, TN, NT):
    """Build + compile the 3-layer SPMD program for per-core capacity C.

    Layer-1 matmuls must consume the 16 k-chunks of the input as they stream
    in, so the m loop runs in pairs (6 PSUM tiles live per pair, 8 banks
    total): each pair's k-loop trickles behind the input DMA instead of one
    m-tile waiting for the entire input. Weight stripes ride the scalar
    (qActDynamicHW) DMA ring so they never queue behind the big input
    transfers on the sync (qSPDynamicHW) ring."""
    widths = [TN] * (NT - 1) + [C - (NT - 1) * TN]
    starts = [i * TN for i in range(NT)]

    nc = bacc.Bacc(
        "TRN2",
        target_bir_lowering=False,
        debug=False,
        enable_asserts=False,
        num_devices=N_CORES,
    )
    xT = nc.dram_tensor("xT", [D, C], F16, kind="ExternalInput").ap()
    w0 = nc.dram_tensor("w0", [MT, PT, D], F16, kind="ExternalInput").ap()
    w1 = nc.dram_tensor("w1", [MT, PT, D], F16, kind="ExternalInput").ap()
    w2 = nc.dram_tensor("w2", [MT, PT, D], F16, kind="ExternalInput").ap()
    bias = nc.dram_tensor("bias", [PT, 3 * MT], F32, kind="ExternalInput").ap()
    yT = nc.dram_tensor("yT", [D, C], F16, kind="ExternalOutput").ap()

    with tile.TileContext(nc) as tc, ExitStack() as ctx:
        wpool = ctx.enter_context(tc.tile_pool(name="w", bufs=4))
        hpool = ctx.enter_context(tc.tile_pool(name="h", bufs=1))
        pspool = ctx.enter_context(tc.tile_pool(name="ps", bufs=8, space="PSUM"))
        opool = ctx.enter_context(tc.tile_pool(name="o", bufs=8))
        cpool = ctx.enter_context(tc.tile_pool(name="c", bufs=1))

        hA = hpool.tile([PT, KT, C], F16, tag="hA")
        hB = hpool.tile([PT, KT, C], F16, tag="hB")

        # All early DMAs round-robin across the shared SDMA engines at packet
        # granularity, so emission order ~= bandwidth share. The first matmul
        # needs stripe (w0, m=0) k=0 + x chunk 0; the pair-0 k-loop then
        # consumes (m0,k)+(m1,k) every ~0.9us, while x chunks land every
        # ~1.9us. Stream the two stripes as interleaved k-range slices so
        # stripe m=1's early k-slices don't queue behind all of stripe m=0
        # (a full stripe is ~3.8us of queue time; a late m1 k=1 slice stalls
        # the PE at the start of the k-loop).
        wts0 = []
        for m in (0, 1):
            wt = wpool.tile([PT, D], F16, tag="wt", name=f"wt0_{m}")
            nc.scalar.dma_start(wt[:, 0:PT], w0[m, :, 0:PT])
            wts0.append(wt)
        nc.sync.dma_start(hA[:, 0, 0:TN], xT[0:PT, 0:TN])
        for ks in range(1, KT, 4):  # k-groups 1-4, 5-8, 9-12, 13-15
            ke = min(ks + 4, KT)
            for m in (0, 1):
                nc.scalar.dma_start(
                    wts0[m][:, ks * PT : ke * PT], w0[m, :, ks * PT : ke * PT]
                )
        if TN < C:
            nc.sync.dma_start(hA[:, 0, TN:C], xT[0:PT, TN:C])
        # x rides the sync ring alone, in chunk order: the SDMA pool is
        # shared (~280 GB/s aggregate per core), so splitting x across rings
        # only adds arrival skew — a single queue already saturates the
        # pool's share and delivers chunks in consumption order.
        for k in range(1, KT):
            nc.sync.dma_start(hA[:, k, :], xT[k * PT : (k + 1) * PT, :])
        bias_sb = cpool.tile([PT, 3 * MT], F32)
        nc.scalar.dma_start(bias_sb[:], bias[:])

        def relu_bias(out_ap, ps_ap, b_ap, on_dve):
            if on_dve:
                nc.vector.tensor_scalar(
                    out_ap, ps_ap, b_ap, 0.0,
                    mybir.AluOpType.add, mybir.AluOpType.max,
                )
            else:
                nc.scalar.activation(
                    out_ap, ps_ap,
                    mybir.ActivationFunctionType.Relu, bias=b_ap,
                )

        layers = [(w0, 0, hA, hB), (w1, 1, hB, hA), (w2, 2, hA, None)]
        for w_dram, li, h_in, h_out in layers:
            for mp in range(MT // 2):
                ms = (2 * mp, 2 * mp + 1)
                if li == 0 and mp == 0:
                    wts = wts0
                else:
                    wts = []
                    halves = 2 if (li == 0 and mp == 1) else 1
                    for m in ms:
                        wt = wpool.tile([PT, D], F16, tag="wt", name=f"wt{li}_{m}")
                        wts.append(wt)
                    # pair 1 of layer 0 still races the pair-0 k-loop;
                    # interleave its two stripes as halves.
                    for hv in range(halves):
                        lo = hv * (D // halves)
                        hi = lo + D // halves
                        for mi, m in enumerate(ms):
                            nc.scalar.dma_start(
                                wts[mi][:, lo:hi], w_dram[m, :, lo:hi]
                            )
                pss = {
                    (m, n): pspool.tile([PT, TN], F32, tag="ps", name=f"ps{li}_{m}_{n}")
                    for m in ms
                    for n in range(NT)
                }

                def epilogue(mi, m, n):
                    n0, w = starts[n], widths[n]
                    b_ap = bias_sb[:, li * MT + m : li * MT + m + 1]
                    # alternate ACT/DVE so epilogues drain on two engines
                    on_dve = (n + mi) % 2 == 1
                    if h_out is not None:
                        relu_bias(
                            h_out[:, m, n0 : n0 + w], pss[(m, n)][:, :w],
                            b_ap, on_dve,
                        )
                    else:
                        # final layer: emit fp16 output tiles (harness
                        # tolerance is 2e-2; fp16 adds ~2e-4 and halves the
                        # output DMA, shortening the post-matmul tail)
                        ot = opool.tile([PT, w], F16, tag="ot", name=f"ot{m}_{n}")
                        relu_bias(ot[:], pss[(m, n)][:, :w], b_ap, on_dve)
                        if m == MT - 1 and n == NT - 1:
                            # very last tile: one epilogue op (splitting it
                            # false-serializes on the shared PSUM tile), then
                            # two half DMAs on separate rings so the final
                            # transfer is half-depth
                            h1 = w // 2
                            nc.scalar.dma_start(
                                yT[m * PT : (m + 1) * PT, n0 : n0 + h1],
                                ot[:, :h1],
                            )
                            nc.sync.dma_start(
                                yT[m * PT : (m + 1) * PT, n0 + h1 : n0 + w],
                                ot[:, h1:w],
                            )
                        else:
                            dma_eng = nc.sync if on_dve else nc.scalar
                            dma_eng.dma_start(
                                yT[m * PT : (m + 1) * PT, n0 : n0 + w], ot[:]
                            )

                if li == 0:
                    # k-outer: consume the streaming input chunks as they land
                    for k in range(KT):
                        for mi, m in enumerate(ms):
                            for n in range(NT):
                                n0, w = starts[n], widths[n]
                                nc.tensor.matmul(
                                    pss[(m, n)][:, :w],
                                    wts[mi][:, k * PT : (k + 1) * PT],
                                    h_in[:, k, n0 : n0 + w],
                                    start=(k == 0),
                                    stop=(k == KT - 1),
                                    skip_group_check=True,
                                )
                    for mi, m in enumerate(ms):
                        for n in range(NT):
                            epilogue(mi, m, n)
                else:
                    # inputs resident: k-inner per tile, so each tile's
                    # epilogue (and final-layer out-DMA) fires as soon as its
                    # accumulation completes — the kernel tail drains one
                    # tile, not six
                    for mi, m in enumerate(ms):
                        for n in range(NT):
                            n0, w = starts[n], widths[n]
                            for k in range(KT):
                                nc.tensor.matmul(
                                    pss[(m, n)][:, :w],
                                    wts[mi][:, k * PT : (k + 1) * PT],
                                    h_in[:, k, n0 : n0 + w],
                                    start=(k == 0),
                                    stop=(k == KT - 1),
                                )
                            epilogue(mi, m, n)
    nc.compile()
    return nc


def _apportion_cores(counts):
    """Assign 8 cores to 4 leaves ~proportionally to token counts.
    Returns list of core counts per leaf (sums to N_CORES; 0 only for empty
    leaves). Greedy: repeatedly hand a core to the leaf with max load/core."""
    alive = [l for l in range(4) if counts[l] > 0]
    n = {l: 1 for l in alive}
    for _ in range(N_CORES - len(alive)):
        l = max(alive, key=lambda l: counts[l] / n[l])
        n[l] += 1
    return [n.get(l, 0) for l in range(4)]


def kernel(x, W0, b0, W1, b1, W2, b2, path_mask):
    global last_results
    x = np.asarray(x, dtype=np.float32)
    path_mask = np.asarray(path_mask)
    W0, b0, W1, b1, W2, b2 = (
        np.asarray(a, dtype=np.float32) for a in (W0, b0, W1, b1, W2, b2)
    )
    B = x.shape[0]

    bit0 = path_mask[:, 0].astype(np.int64)
    bit1 = path_mask[:, 1].astype(np.int64)
    leaf = 2 * bit0 + bit1
    order = np.argsort(leaf, kind="stable")
    counts = np.bincount(leaf, minlength=4)

    per_leaf = _apportion_cores(counts)
    # contiguous chunks of the leaf-sorted order per core
    groups = []      # list of (leaf, index-array) per core
    start = 0
    for l in range(4):
        cnt = int(counts[l])
        tok = order[start : start + cnt]
        start += cnt
        nl = per_leaf[l]
        if nl == 0:
            continue
        bounds = [round(i * cnt / nl) for i in range(nl + 1)]
        for i in range(nl):
            groups.append((l, tok[bounds[i] : bounds[i + 1]]))
    while len(groups) < N_CORES:  # only if some leaf was empty and slots remain
        groups.append((0, np.zeros(0, dtype=np.int64)))

    maxg = max(len(g[1]) for g in groups)
    TN, NT, C = _tiling(maxg)

    if C not in _compiled:
        _compiled[C] = _build(C, TN, NT)
    nc = _compiled[C]

    w_prepped = {}  # cache per (matrix id)
    def wp(tag, W):
        if tag not in w_prepped:
            w_prepped[tag] = _prep_weight(W)
        return w_prepped[tag]

    xb = x.astype(NP_F16)
    in_maps = []
    for l, tok in groups:
        xTg = np.zeros((D, C), dtype=NP_F16)
        if len(tok):
            xTg[:, : len(tok)] = xb[tok].T
        in_maps.append(
            {
                "xT": xTg,
                "w0": wp("w0", W0),
                "w1": wp(("w1", l // 2), W1[l // 2]),
                "w2": wp(("w2", l), W2[l]),
                "bias": _prep_bias(b0, b1[l // 2], b2[l]),
            }
        )

    last_results = run_bass_kernel_spmd(nc, in_maps, core_ids=list(range(N_CORES)))

    y = np.empty((B, D), dtype=np.float32)
    for (l, tok), res in zip(groups, last_results.results):
        if len(tok):
            y[tok] = res["yT"][:, : len(tok)].T.astype(np.float32)
    return y


# revision 22
# speedup vs baseline: 1.0224x; 1.0037x over previous
"""Binary-tree gated-expert MoE (root -> 2 mid -> 4 leaf experts) on 8 trn2 cores.

Strategy: expert-parallel dispatch by leaf index. Tokens are grouped on the
host by their 2-bit routing path (leaf = 2*bit0 + bit1); each of the 8
NeuronCores processes one contiguous chunk of one leaf's tokens (cores are
apportioned to leaves proportionally to token counts, 2 cores/leaf in the
balanced case). A core then runs 3 chained dense [C,2048]x[2048,2048] layers
(root W0, mid W1[bit0], leaf W2[leaf]) with relu+bias, entirely on-chip.

Device kernel keeps activations transposed ([D, tokens] feature-major) so each
layer's matmul output (PSUM [fout, tok]) is directly the next layer's rhs.
Matmuls run in fp16 (same TensorE rate as bf16, 8x finer mantissa) with fp32
PSUM accumulation; weights are streamed from HBM as pre-tiled [16, 128, 2048]
stripes and used as the stationary operand.

Schedule notes (from NTFF trace analysis): the kernel is TensorE-bound at
~1.02 cycles/column, so the only wins are at the edges. The first m-pair's
weight stripes stream as interleaved k-range slices so the pair-0 k-loop
(which trickles behind the x input DMA) never waits on stripe m=1; the output
is fp16 (harness tolerance 2e-2, fp16 adds ~2e-4) to halve output DMA; and
the last m-pair's epilogues are chunked column-wise across ACT/DVE and both
DMA rings so the post-matmul tail is short.
"""

import numpy as np
from contextlib import ExitStack

import concourse.bass as bass
from concourse import bacc, mybir, tile
from concourse.bass_utils import run_bass_kernel_spmd

D = 2048
PT = 128           # partition tile
KT = D // PT       # 16 contraction tiles per layer
MT = D // PT       # 16 output-feature tiles per layer
N_CORES = 8

F32 = mybir.dt.float32
F16 = mybir.dt.float16
NP_F16 = np.float16

# cache of compiled bass programs keyed by padded capacity C
_compiled = {}
# stash of the last run's results so a harness can inspect exec_time_ns
last_results = None


def _prep_weight(W):
    """[D, D] -> [MT, 128, D] fp16: stripe m holds W[:, m*128:(m+1)*128]
    rearranged so partition p = contraction row within k-chunk, and the free
    dim is (k, fout-col) — i.e. out[m, p, k*128 + c] = W[k*128 + p, m*128 + c].
    Each [128, 2048] stripe then DMAs contiguously into SBUF and its k-th
    [128, 128] column block is exactly the lhsT (stationary) matmul operand."""
    W4 = W.reshape(KT, PT, MT, PT)
    return np.ascontiguousarray(
        W4.transpose(2, 1, 0, 3).reshape(MT, PT, D).astype(NP_F16)
    )


def _prep_bias(b0, b1e, b2l):
    """three [D] biases -> [128, 3*MT] f32 where column li*MT + m holds
    bias[li][m*128 : (m+1)*128] along partitions."""
    cols = []
    for b in (b0, b1e, b2l):
        cols.append(b.reshape(MT, PT).T)  # [128, MT]
    return np.ascontiguousarray(np.concatenate(cols, axis=1).astype(np.float32))


def _tiling(maxg):
    """Pick (TN, NT, C): NT token tiles, first NT-1 of width TN (<= 512, one
    PSUM bank of fp32) and a final tile of C - (NT-1)*TN, with C = maxg
    exactly (no padding beyond the max per-core group size)."""
    maxg = max(maxg, 256)
    NT = -(-maxg // 512)
    TN = -(-maxg // NT)
    return TN, NT, maxg


def _build(C, TN, NT):
    """Build + compile the 3-layer SPMD program for per-core capacity C.

    Layer-1 matmuls must consume the 16 k-chunks of the input as they stream
    in, so the m loop runs in pairs (6 PSUM tiles live per pair, 8 banks
    total): each pair's k-loop trickles behind the input DMA instead of one
    m-tile waiting for the entire input. Weight stripes ride the scalar
    (qActDynamicHW) DMA ring so they never queue behind the big input
    transfers on the sync (qSPDynamicHW) ring."""
    widths = [TN] * (NT - 1) + [C - (NT - 1) * TN]
    starts = [i * TN for i in range(NT)]

    nc = bacc.Bacc(
        "TRN2",
        target_bir_lowering=False,
        debug=False,
        enable_asserts=False,
        num_devices=N_CORES,
    )
    xT = nc.dram_tensor("xT", [D, C], F16, kind="ExternalInput").ap()
    w0 = nc.dram_tensor("w0", [MT, PT, D], F16, kind="ExternalInput").ap()
    w1 = nc.dram_tensor("w1", [MT, PT, D], F16, kind="ExternalInput").ap()
    w2 = nc.dram_tensor("w2", [MT, PT, D], F16, kind="ExternalInput").ap()
    bias = nc.dram_tensor("bias", [PT, 3 * MT], F32, kind="ExternalInput").ap()
    yT = nc.dram_tensor("yT", [D, C], F16, kind="ExternalOutput").ap()

    with tile.TileContext(nc) as tc, ExitStack() as ctx:
        wpool = ctx.enter_context(tc.tile_pool(name="w", bufs=3))
        hpool = ctx.enter_context(tc.tile_pool(name="h", bufs=1))
        pspool = ctx.enter_context(tc.tile_pool(name="ps", bufs=8, space="PSUM"))
        opool = ctx.enter_context(tc.tile_pool(name="o", bufs=8))
        cpool = ctx.enter_context(tc.tile_pool(name="c", bufs=1))

        hA = hpool.tile([PT, KT, C], F16, tag="hA")
        hB = hpool.tile([PT, KT, C], F16, tag="hB")

        # All early DMAs round-robin across the shared SDMA engines at packet
        # granularity, so emission order ~= bandwidth share. The first matmul
        # needs stripe (w0, m=0) k=0 + x chunk 0; the pair-0 k-loop then
        # consumes (m0,k)+(m1,k) every ~0.9us, while x chunks land every
        # ~1.9us. Stream the two stripes as interleaved k-range slices so
        # stripe m=1's early k-slices don't queue behind all of stripe m=0
        # (a full stripe is ~3.8us of queue time; a late m1 k=1 slice stalls
        # the PE at the start of the k-loop).
        wts0 = []
        for m in (0, 1):
            wt = wpool.tile([PT, D], F16, tag="wt", name=f"wt0_{m}")
            nc.scalar.dma_start(wt[:, 0:PT], w0[m, :, 0:PT])
            wts0.append(wt)
        nc.sync.dma_start(hA[:, 0, 0:TN], xT[0:PT, 0:TN])
        for ks in range(1, KT, 4):  # k-groups 1-4, 5-8, 9-12, 13-15
            ke = min(ks + 4, KT)
            for m in (0, 1):
                nc.scalar.dma_start(
                    wts0[m][:, ks * PT : ke * PT], w0[m, :, ks * PT : ke * PT]
                )
        if TN < C:
            nc.sync.dma_start(hA[:, 0, TN:C], xT[0:PT, TN:C])
        # x rides the sync ring alone, in chunk order: the SDMA pool is
        # shared (~280 GB/s aggregate per core), so splitting x across rings
        # only adds arrival skew — a single queue already saturates the
        # pool's share and delivers chunks in consumption order.
        for k in range(1, KT):
            nc.sync.dma_start(hA[:, k, :], xT[k * PT : (k + 1) * PT, :])
        bias_sb = cpool.tile([PT, 3 * MT], F32)
        nc.scalar.dma_start(bias_sb[:], bias[:])

        def relu_bias(out_ap, ps_ap, b_ap, on_dve):
            if on_dve:
                nc.vector.tensor_scalar(
                    out_ap, ps_ap, b_ap, 0.0,
                    mybir.AluOpType.add, mybir.AluOpType.max,
                )
            else:
                nc.scalar.activation(
                    out_ap, ps_ap,
                    mybir.ActivationFunctionType.Relu, bias=b_ap,
                )

        def epilogue(li, h_out, m, n, ps):
            n0, w = starts[n], widths[n]
            b_ap = bias_sb[:, li * MT + m : li * MT + m + 1]
            # alternate ACT/DVE so epilogues drain on two engines
            on_dve = (n + m) % 2 == 1
            if h_out is not None:
                relu_bias(h_out[:, m, n0 : n0 + w], ps[:, :w], b_ap, on_dve)
            else:
                # final layer: emit fp16 output tiles (harness tolerance is
                # 2e-2; fp16 adds ~2e-4 and halves the output DMA,
                # shortening the post-matmul tail)
                ot = opool.tile([PT, w], F16, tag="ot", name=f"ot{m}_{n}")
                relu_bias(ot[:], ps[:, :w], b_ap, on_dve)
                if m == MT - 1 and n == NT - 1:
                    # very last tile: one epilogue op (splitting it
                    # false-serializes on the shared PSUM tile), then two
                    # half DMAs on separate rings so the final transfer is
                    # half-depth
                    h1 = w // 2
                    nc.scalar.dma_start(
                        yT[m * PT : (m + 1) * PT, n0 : n0 + h1], ot[:, :h1]
                    )
                    nc.sync.dma_start(
                        yT[m * PT : (m + 1) * PT, n0 + h1 : n0 + w], ot[:, h1:w]
                    )
                else:
                    dma_eng = nc.sync if on_dve else nc.scalar
                    dma_eng.dma_start(
                        yT[m * PT : (m + 1) * PT, n0 : n0 + w], ot[:]
                    )

        # ── layer 0, m-pair (0,1): k-outer, consuming x chunks as they
        # stream in. Everything after runs m-serial k-inner.
        pss0 = {
            (m, n): pspool.tile([PT, TN], F32, tag="ps", name=f"ps0_{m}_{n}")
            for m in (0, 1)
            for n in range(NT)
        }
        for k in range(KT):
            for mi, m in enumerate((0, 1)):
                for n in range(NT):
                    n0, w = starts[n], widths[n]
                    nc.tensor.matmul(
                        pss0[(m, n)][:, :w],
                        wts0[mi][:, k * PT : (k + 1) * PT],
                        hA[:, k, n0 : n0 + w],
                        start=(k == 0),
                        stop=(k == KT - 1),
                        skip_group_check=True,
                    )
        for m in (0, 1):
            for n in range(NT):
                epilogue(0, hB, m, n, pss0[(m, n)])

        # ── remaining m-tiles of layer 0 + layers 1,2: m-serial, k-inner.
        # x is fully resident once pair-0 ends (its k=15 gates on the last
        # chunk), so k-inner is safe and each (m,n) epilogue fires as soon
        # as its accumulation completes. One stripe tile per m with a
        # 3-deep pool: stripe m_j's DMA descriptor carries a wait on
        # m_{j-3}'s readers, so beyond m2 the stripes self-pace one m ahead
        # of the PE instead of flooding the early window that the x stream
        # needs (pair-0 is x-delivery-bound).
        schedule = [(0, hA, hB, w0, m) for m in range(2, MT)]
        for li, h_in, h_out, w_dram in ((1, hB, hA, w1), (2, hA, None, w2)):
            schedule += [(li, h_in, h_out, w_dram, m) for m in range(MT)]
        for li, h_in, h_out, w_dram, m in schedule:
            wt = wpool.tile([PT, D], F16, tag="wt", name=f"wt{li}_{m}")
            nc.scalar.dma_start(wt[:], w_dram[m])
            for n in range(NT):
                n0, w = starts[n], widths[n]
                ps = pspool.tile([PT, TN], F32, tag="ps", name=f"ps{li}_{m}_{n}")
                for k in range(KT):
                    nc.tensor.matmul(
                        ps[:, :w],
                        wt[:, k * PT : (k + 1) * PT],
                        h_in[:, k, n0 : n0 + w],
                        start=(k == 0),
                        stop=(k == KT - 1),
                    )
                epilogue(li, h_out, m, n, ps)
    nc.compile()
    return nc


def _apportion_cores(counts):
    """Assign 8 cores to 4 leaves ~proportionally to token counts.
    Returns list of core counts per leaf (sums to N_CORES; 0 only for empty
    leaves). Greedy: repeatedly hand a core to the leaf with max load/core."""
    alive = [l for l in range(4) if counts[l] > 0]
    n = {l: 1 for l in alive}
    for _ in range(N_CORES - len(alive)):
        l = max(alive, key=lambda l: counts[l] / n[l])
        n[l] += 1
    return [n.get(l, 0) for l in range(4)]


def kernel(x, W0, b0, W1, b1, W2, b2, path_mask):
    global last_results
    x = np.asarray(x, dtype=np.float32)
    path_mask = np.asarray(path_mask)
    W0, b0, W1, b1, W2, b2 = (
        np.asarray(a, dtype=np.float32) for a in (W0, b0, W1, b1, W2, b2)
    )
    B = x.shape[0]

    bit0 = path_mask[:, 0].astype(np.int64)
    bit1 = path_mask[:, 1].astype(np.int64)
    leaf = 2 * bit0 + bit1
    order = np.argsort(leaf, kind="stable")
    counts = np.bincount(leaf, minlength=4)

    per_leaf = _apportion_cores(counts)
    # contiguous chunks of the leaf-sorted order per core
    groups = []      # list of (leaf, index-array) per core
    start = 0
    for l in range(4):
        cnt = int(counts[l])
        tok = order[start : start + cnt]
        start += cnt
        nl = per_leaf[l]
        if nl == 0:
            continue
        bounds = [round(i * cnt / nl) for i in range(nl + 1)]
        for i in range(nl):
            groups.append((l, tok[bounds[i] : bounds[i + 1]]))
    while len(groups) < N_CORES:  # only if some leaf was empty and slots remain
        groups.append((0, np.zeros(0, dtype=np.int64)))

    maxg = max(len(g[1]) for g in groups)
    TN, NT, C = _tiling(maxg)

    if C not in _compiled:
        _compiled[C] = _build(C, TN, NT)
    nc = _compiled[C]

    w_prepped = {}  # cache per (matrix id)
    def wp(tag, W):
        if tag not in w_prepped:
            w_prepped[tag] = _prep_weight(W)
        return w_prepped[tag]

    xb = x.astype(NP_F16)
    in_maps = []
    for l, tok in groups:
        xTg = np.zeros((D, C), dtype=NP_F16)
        if len(tok):
            xTg[:, : len(tok)] = xb[tok].T
        in_maps.append(
            {
                "xT": xTg,
                "w0": wp("w0", W0),
                "w1": wp(("w1", l // 2), W1[l // 2]),
                "w2": wp(("w2", l), W2[l]),
                "bias": _prep_bias(b0, b1[l // 2], b2[l]),
            }
        )

    last_results = run_bass_kernel_spmd(nc, in_maps, core_ids=list(range(N_CORES)))

    y = np.empty((B, D), dtype=np.float32)
    for (l, tok), res in zip(groups, last_results.results):
        if len(tok):
            y[tok] = res["yT"][:, : len(tok)].T.astype(np.float32)
    return y
